# revision 1
# baseline (speedup 1.0000x reference)
"""Multi-head causal attention (B=4, T=1024, C=1024, H=16, D=64) on 8 TRN2 cores.

Sharding: tensor-parallel over heads. Core i owns heads {2i, 2i+1}:
  - x is replicated (sent pre-transposed as xT [C, B*T], bf16)
  - Wq/Wk/Wv sharded over heads -> per-core [C, 128] (2 heads concat on D)
  - row-parallel output projection: per-core Wp rows [128, C]; host sums the
    8 partial [B*T, C] outputs (the all-reduce) and adds bp.

Scheduling rewrite (175us baseline -> ~150us): the PE stream is kept as
gap-free as possible so the HAM clock gate stays at 2.4 GHz (idle gaps over
~1us re-throttle the PE to 1.2 GHz for multiples of the 3.4us window):
  - 20 warm-up junk matmuls cover the input-DMA lead-in
  - x is bulk-loaded once for all batches (contiguous 8KB-row descriptors,
    batch-0 slices first); weights arrive host-pre-shuffled so each weight
    is one contiguous [128, 1024] DMA
  - per batch: q,k matmuls, scores s=0/1 early (ACT exp head start), then
    v/transposes + a lag-2 prev proj half interleaved into the exp-bound
    scores loop as PE backfill; po1 runs h-outer
  - scores psums alternate ps512/psvt pools (4-bank rotation = exp-backlog
    elasticity); last batch's po1 lives in the idle proj pool so it never
    waits on the normalize bounce; flush rotates ps512+psvt
  - softmax normalize: denominators from the ones-column of the attnV
    stationary; packed [32,32] reciprocal via a DRAM bounce (coarse DMA
    descriptors); the tail normalize hides under a 4-half proj flush
  - one merged out-DMA per proj half ([128,4,1024]) keeps the Sync engine's
    ~600ns-per-DMA DIRECT2D dispatch off the critical path
  - elementwise balance: ACT=exp+den+vT+1/4 proj copies, DVE=qk/vaug
    copies+recip+normalize muls+3/4 proj copies, GPSIMD=masks (GPSIMD
    cannot touch PSUM)
"""

import ml_dtypes
import numpy as np

B, T, C = 4, 1024, 1024
H, D = 16, 64
NCORES = 8
HPC = H // NCORES      # heads per core = 2
D2 = HPC * D           # 128
BT = B * T
SCALE = 1.0 / np.sqrt(np.float32(C))  # 1/32
BF16 = ml_dtypes.bfloat16

_compiled = None

NWARM = 20


def _split_multi_waits(nc, mybir, maxw=1):
    """Walrus in this container encodes at most one sync wait per
    instruction (fp32 self-loading matmuls and drains overflow).  Hoist
    excess waits onto same-engine NoOps inserted just before."""
    for fn in nc.m.functions:
        for bb in fn.blocks:
            new = []
            for inst in bb.instructions:
                si = inst.sync_info
                waits = list(si.on_wait) if (si is not None and si.on_wait) else []
                if len(waits) > maxw:
                    extra, keep = waits[:-maxw], waits[-maxw:]
                    for j, w in enumerate(extra):
                        new.append(
                            mybir.InstNoOp(
                                name=f"{inst.name}-wsplit{j}",
                                engine=inst.engine,
                                sync_info=mybir.SyncInfo(on_wait=[w], on_update=[]),
                                bass_nofuse=True,
                            )
                        )
                    inst.sync_info = mybir.SyncInfo(
                        on_wait=keep,
                        on_update=list(si.on_update) if si.on_update else [],
                    )
                new.append(inst)
            bb.instructions = new


def _build():
    import concourse.bass as bass
    import concourse.mybir as mybir
    import concourse.tile as tile

    f32 = mybir.dt.float32
    bf = mybir.dt.bfloat16
    EXP = mybir.ActivationFunctionType.Exp

    nc = bass.Bass("TRN2", target_bir_lowering=False, debug=False, num_devices=NCORES)

    xT_d = nc.dram_tensor("xT", [C, BT], bf, kind="ExternalInput").ap()
    # host pre-shuffles each weight to [p, k, m] so the DMA is contiguous
    wq_d = nc.dram_tensor("wq", [128, C // 128, D2], bf, kind="ExternalInput").ap()
    wk_d = nc.dram_tensor("wk", [128, C // 128, D2], bf, kind="ExternalInput").ap()
    wv_d = nc.dram_tensor("wv", [128, C // 128, D2], bf, kind="ExternalInput").ap()
    wp_d = nc.dram_tensor("wp", [D2, C], bf, kind="ExternalInput").ap()
    mask_d = nc.dram_tensor("mask", [128, 128], bf, kind="ExternalInput").ap()
    ident_d = nc.dram_tensor("ident", [128, 128], bf, kind="ExternalInput").ap()
    out_d = nc.dram_tensor("out", [BT, C], bf, kind="ExternalOutput").ap()

    KC = C // 128  # 8 contraction chunks over C
    NS = T // 128  # 8 s-chunks
    NH = 2         # two 512-wide t halves

    import concourse.bass as _bass

    with tile.TileContext(nc) as tc:
        with (
            tc.tile_pool(name="const", bufs=1) as constp,
            tc.tile_pool(name="xin", bufs=1) as xinp,
            tc.tile_pool(name="qkv", bufs=2) as qkvp,
            tc.tile_pool(name="vaug", bufs=2) as vaugp,
            tc.tile_pool(name="exps", bufs=18) as expp,
            tc.tile_pool(name="smalls", bufs=4) as smallp,
            tc.tile_pool(name="outt", bufs=3) as outtp,
            tc.tile_pool(name="pout", bufs=4) as poutp,
            tc.tile_pool(name="dram", bufs=2, space="DRAM") as dramp,
            tc.tile_pool(name="ps512", bufs=2, space="PSUM") as ps512,
            tc.tile_pool(name="psatt", bufs=2, space="PSUM") as psatt,
            tc.tile_pool(name="psvt", bufs=2, space="PSUM") as psvt,
            tc.tile_pool(name="psproj", bufs=2, space="PSUM") as psproj,
        ):
            # ---- constants / warmup ----
            wq_s = constp.tile([128, KC, D2], bf, tag="wq")
            wk_s = constp.tile([128, KC, D2], bf, tag="wk")
            wv_s = constp.tile([128, KC, D2], bf, tag="wv")
            wp_s = constp.tile([128, C], bf, tag="wp")
            mask_s = constp.tile([128, 128], bf, tag="mask")
            ident = constp.tile([128, 128], bf, tag="ident")
            junk = constp.tile([128, 512], bf, tag="junk")

            # Warm-up: junk matmuls with no DMA deps fill the PE stream while
            # inputs land, so HAM un-throttles before real work and never
            # re-throttles (any later stall is << the 3.4us MID window).
            nc.vector.memset(junk[:], 0.0)
            for i in range(NWARM):
                pw = ps512.tile([128, 512], f32, tag="ps512", name=f"warm{i}")
                nc.tensor.matmul(pw[:], junk[:, 0:128], junk[:], start=True, stop=True)

            # Input DMAs, critical-first: wq + the first x chunk gate the
            # first real matmul.  x is loaded in one shot for all batches
            # (k-chunk DMAs of contiguous 8KB rows - 128 descriptors each);
            # weights arrive pre-shuffled from the host so each is a single
            # contiguous [128, 1024] transfer.
            xba = xinp.tile([128, KC, BT], bf, tag="xba", name="xba", bufs=1)
            nc.sync.dma_start(wq_s[:], wq_d)
            for k in range(KC):
                nc.sync.dma_start(
                    xba[:, k, 0:T], xT_d[k * 128:(k + 1) * 128, 0:T]
                )
                if k == 0:
                    nc.sync.dma_start(wk_s[:], wk_d)
                if k == 1:
                    nc.sync.dma_start(wv_s[:], wv_d)
            for k in range(KC):
                nc.sync.dma_start(
                    xba[:, k, T:BT], xT_d[k * 128:(k + 1) * 128, T:BT]
                )
            nc.sync.dma_start(mask_s[:], mask_d)
            nc.sync.dma_start(ident[:], ident_d)
            nc.sync.dma_start(wp_s[:], wp_d)

            def emit_qk(b, xb):
                qT = qkvp.tile([128, T], bf, tag="qT", name=f"qT{b}")
                kT = qkvp.tile([128, T], bf, tag="kT", name=f"kT{b}")
                for w_s, oT in ((wq_s, qT), (wk_s, kT)):
                    for half in range(NH):
                        ps = ps512.tile([128, 512], f32, tag="ps512")
                        for k in range(KC):
                            nc.tensor.matmul(
                                ps[:],
                                w_s[:, k, :],
                                xb[:, k, half * 512:(half + 1) * 512],
                                start=(k == 0),
                                stop=(k == KC - 1),
                            )
                        nc.vector.tensor_copy(
                            oT[:, half * 512:(half + 1) * 512], ps[:]
                        )
                return qT, kT

            def emit_v_mms(b, xb, half, vT):
                ps = ps512.tile([128, 512], f32, tag="ps512")
                for k in range(KC):
                    nc.tensor.matmul(
                        ps[:],
                        wv_s[:, k, :],
                        xb[:, k, half * 512:(half + 1) * 512],
                        start=(k == 0),
                        stop=(k == KC - 1),
                    )
                nc.scalar.copy(vT[:, half * 512:(half + 1) * 512], ps[:])

            def emit_vtrans(b, vT, vaug, srange):
                for s in srange:
                    pv = psvt.tile([128, 128], bf, tag="psvt")
                    nc.tensor.transpose(
                        pv[:], vT[:, s * 128:(s + 1) * 128], ident[:]
                    )
                    nc.vector.tensor_copy(vaug[:, s, :, 0:64], pv[:])

            # Scores psums alternate between the two pools: transposes are
            # idle during the s-loop, so this gives a 4-bank rotation and the
            # ACT exp backlog ~1.5us of elasticity before PE stalls on a bank.
            _sctr = [0]

            def score_ps(force=None):
                _sctr[0] += 1
                sel = (_sctr[0] % 2) if force is None else force
                pool, tag = ((ps512, "ps512") if sel else (psvt, "psvt"))
                return pool.tile(
                    [128, 512], f32, tag=tag, name=f"sc{_sctr[0]}"
                )

            def emit_scores_s(b, s, qT, kT, exs):
                s0 = s * 128
                d1 = max(0, s0 - 512)
                for h in range(HPC):
                    hp = slice(h * 64, (h + 1) * 64)
                    ex = expp.tile(
                        [128, 1024], bf, tag="ex", bufs=18, name=f"ex{b}_{h}_{s}"
                    )
                    exs[(h, s)] = ex
                    if s < 4:  # t-half0 piece: cols [s0, 512)
                        w0 = 512 - s0
                        pa = score_ps()
                        nc.tensor.matmul(
                            pa[:, 0:w0],
                            kT[hp, s0:s0 + 128],
                            qT[hp, s0:512],
                            start=True,
                            stop=True,
                        )
                        nc.scalar.activation(
                            ex[:, 0:w0], pa[:, 0:w0], EXP, scale=float(SCALE)
                        )
                    # t-half1 piece: cols [max(512, s0), 1024)
                    w1 = 512 - d1
                    pb = score_ps()
                    nc.tensor.matmul(
                        pb[:, 0:w1],
                        kT[hp, s0:s0 + 128],
                        qT[hp, 512 + d1:T],
                        start=True,
                        stop=True,
                    )
                    nc.scalar.activation(
                        ex[:, 512 - s0 + d1:T - s0],
                        pb[:, 0:w1],
                        EXP,
                        scale=float(SCALE),
                    )
                    nc.gpsimd.tensor_mul(ex[:, 0:128], ex[:, 0:128], mask_s[:])

            def emit_po0_s(b, s, vaug, exs, po0):
                assert 0 <= s <= 3
                s0 = s * 128
                for h in range(HPC):
                    nc.tensor.matmul(
                        po0[h][0:65, s0:512],
                        vaug[:, s, h, 0:65],
                        exs[(h, s)][:, 0:512 - s0],
                        start=(s == 0),
                        stop=(s == 3),
                    )

            def emit_normalize_half(b, half, po_h, outT2):
                t0 = half * 512
                den2 = smallp.tile(
                    [1, 2 * 512], f32, tag="den2", bufs=2, name=f"den2_{b}_{half}"
                )
                for h in range(HPC):
                    nc.scalar.copy(
                        den2[0:1, h * 512:(h + 1) * 512], po_h[h][64:65, 0:512]
                    )
                # Packed reciprocal: bounce the 1024 denominators through DRAM
                # to use all 128 DVE lanes, then DMA-broadcast each [64, 512]
                # operand back.
                scr_rec = dramp.tile(
                    [1, 1024], f32, tag="scr_rec", name=f"scrr_{b}_{half}"
                )
                packed = smallp.tile([32, 32], f32, tag="packed")
                nc.sync.dma_start(packed[:], den2[0:1, :])
                recp = smallp.tile([32, 32], f32, tag="recp")
                nc.vector.reciprocal(recp[:], packed[:])
                nc.sync.dma_start(
                    scr_rec[0, :].rearrange("(p f) -> p f", p=32), recp[:]
                )
                for h in range(HPC):
                    hp = slice(h * 64, (h + 1) * 64)
                    rec2 = smallp.tile(
                        [64, 512], f32, tag="rec2", name=f"rec2_{b}_{half}_{h}"
                    )
                    nc.sync.dma_start(
                        rec2[:],
                        _bass.AP(
                            scr_rec[:].tensor,
                            scr_rec[:].offset + 512 * h,
                            [[0, 64], [1, 512]],
                        ),
                    )
                    nc.vector.tensor_mul(
                        outT2[hp, t0:t0 + 512], po_h[h][0:64, 0:512], rec2[:]
                    )

            PROJ_COPY_ENG = ("v", "s", "v", "v")

            _fctr = [0]

            def emit_proj_tile(pb, o2, i, tt, ob, wide=False):
                for ct in range(2):
                    # During the flush (wide=True) the scores/transpose pools
                    # are drained, so cycle all three for a 6-bank rotation.
                    if wide:
                        _fctr[0] += 1
                        pool, tag = [(ps512, "ps512"),
                                     (psvt, "psvt")][_fctr[0] % 2]
                        pp = pool.tile([128, 512], f32, tag=tag,
                                       name=f"fl{_fctr[0]}")
                    else:
                        pp = psproj.tile([128, 512], f32, tag="psproj")
                    nc.tensor.matmul(
                        pp[:],
                        o2[:, tt * 128:(tt + 1) * 128],
                        wp_s[:, ct * 512:(ct + 1) * 512],
                        start=True,
                        stop=True,
                    )
                    if wide:
                        eng = ("v", "s")[(2 * i + ct) % 2]
                    else:
                        eng = PROJ_COPY_ENG[(2 * i + ct) % 4]
                    dst = ob[:, i, ct * 512:(ct + 1) * 512]
                    if eng == "s":
                        nc.scalar.copy(dst, pp[:])
                    else:
                        nc.vector.tensor_copy(dst, pp[:])

            def proj_half_ob(pb, half):
                ob = poutp.tile([128, 4, C], bf, tag="ob",
                                name=f"ob{pb}_{half}")
                return ob

            def emit_proj_out(pb, half, ob):
                r0 = pb * T + half * 512
                nc.sync.dma_start(
                    out_d[r0:r0 + 512, :].rearrange("(t p) c -> p t c", t=4),
                    ob[:],
                )

            def emit_proj_half(pb, o2, half, wide=False):
                ob = proj_half_ob(pb, half)
                for i, tt in enumerate(range(half * 4, half * 4 + 4)):
                    emit_proj_tile(pb, o2, i, tt, ob, wide=wide)
                emit_proj_out(pb, half, ob)

            # ---- main pipeline ----
            pend = []  # pending proj halves: (batch, outT2, half)

            def pop_proj(b):
                # lag-2: only emit proj halves at least two batches old, so
                # the normalize DMA-bounce latency is always covered and the
                # tail flush has three proj halves of PE work in front of the
                # last (normalize-gated) one.  Returns a list of per-tile
                # thunks so callers can spread the tiles into the ACT-bound
                # scores loop as PE backfill.
                if pend and pend[0][0] <= b - 2:
                    pb, o2, half = pend.pop(0)
                    ob = proj_half_ob(pb, half)

                    def mk(i, tt):
                        def thunk():
                            emit_proj_tile(pb, o2, i, tt, ob)
                            if i == 3:
                                emit_proj_out(pb, half, ob)
                        return thunk

                    return [
                        mk(i, tt)
                        for i, tt in enumerate(range(half * 4, half * 4 + 4))
                    ]
                return []

            outs = {}
            for b in range(B):
                xb = xba[:, :, b * T:(b + 1) * T]
                vaug = vaugp.tile([128, NS, HPC, 66], bf, tag="vaug",
                                  name=f"vaug{b}")
                nc.vector.memset(vaug[:, :, :, 64:65], 1.0)
                outT2 = outtp.tile([128, T], bf, tag="outT2", name=f"outT2_{b}")
                outs[b] = outT2
                exs = {}

                qT, kT = emit_qk(b, xb)
                # scores s=0 early: gives ACT exp a head start over the
                # v/proj PE work that follows.
                emit_scores_s(b, 0, qT, kT, exs)
                vT = qkvp.tile([128, T], bf, tag="vT", name=f"vT{b}")
                emit_v_mms(b, xb, 0, vT)
                emit_scores_s(b, 1, qT, kT, exs)
                emit_v_mms(b, xb, 1, vT)
                emit_vtrans(b, vT, vaug, range(0, 4))
                slot_a = pop_proj(b)  # slot A: old proj half, spread out
                for t in slot_a[:2]:
                    t()
                emit_vtrans(b, vT, vaug, range(4, NS))

                po0 = [
                    psatt.tile([128, 512], f32, tag="psatt", name=f"po0_{b}_{h}")
                    for h in range(HPC)
                ]
                for s in range(2, NS):
                    emit_scores_s(b, s, qT, kT, exs)
                    if s - 2 <= 3:
                        emit_po0_s(b, s - 2, vaug, exs, po0)
                    if s - 2 == 3:
                        emit_normalize_half(b, 0, po0, outT2)
                    if s - 2 < len(slot_a) - 2:
                        slot_a[2 + (s - 2)]()  # PE backfill, no exp dep

                po1_pool, po1_tag = (
                    (psproj, "psproj") if b == B - 1 else (psatt, "psatt")
                )
                po1 = [
                    po1_pool.tile([128, 512], f32, tag=po1_tag,
                                  name=f"po1_{b}_{h}")
                    for h in range(HPC)
                ]
                for h in range(HPC):
                    for s in range(NS):
                        s0 = s * 128
                        d1 = max(0, s0 - 512)
                        nc.tensor.matmul(
                            po1[h][0:65, d1:512],
                            vaug[:, s, h, 0:65],
                            exs[(h, s)][:, 512 - s0 + d1:T - s0],
                            start=(s == 0),
                            stop=(s == NS - 1),
                        )
                pend.append((b, outT2, 0))
                emit_normalize_half(b, 1, po1, outT2)
                if b != B - 1:
                    for t in pop_proj(b):  # slot B
                        t()
                pend.append((b, outT2, 1))

            while pend:
                pb, o2, half = pend.pop(0)
                emit_proj_half(pb, o2, half, wide=True)

    _split_multi_waits(nc, mybir)
    return nc


def _get_compiled():
    global _compiled
    if _compiled is None:
        _compiled = _build()
    return _compiled


def _shuf_w(W, h0):
    # [H, C, D] head-pair slice -> [C, D2] -> pre-shuffled [p, k, m] so the
    # device DMA is one contiguous [128, 1024] transfer per weight.
    w = np.asarray(W[h0:h0 + HPC], dtype=np.float32).transpose(1, 0, 2).reshape(C, D2)
    return np.ascontiguousarray(
        w.reshape(C // 128, 128, D2).transpose(1, 0, 2)
    ).astype(BF16)


def _make_in_maps(x, Wq, Wk, Wv, Wp):
    xT = np.ascontiguousarray(
        np.asarray(x, dtype=np.float32).reshape(BT, C).T
    ).astype(BF16)  # [C, BT]
    mask = np.triu(np.ones((128, 128), dtype=BF16))  # keep j >= i
    ident = np.eye(128, dtype=BF16)
    in_maps = []
    for i in range(NCORES):
        h0 = i * HPC
        wp = np.ascontiguousarray(
            np.asarray(Wp, dtype=np.float32)[h0 * D:(h0 + HPC) * D, :]
        ).astype(BF16)
        in_maps.append(
            {"xT": xT, "wq": _shuf_w(Wq, h0), "wk": _shuf_w(Wk, h0),
             "wv": _shuf_w(Wv, h0), "wp": wp, "mask": mask, "ident": ident}
        )
    return in_maps


def run(x, Wq, Wk, Wv, Wp, bp, trace=False, trace_cores=None):
    """Returns (full_output [B,T,C], BassKernelResults)."""
    from concourse.bass_utils import run_bass_kernel_spmd

    nc = _get_compiled()
    in_maps = _make_in_maps(x, Wq, Wk, Wv, Wp)
    kw = {}
    if trace:
        kw = {"trace": True, "trace_cores": trace_cores or [0]}
    res = run_bass_kernel_spmd(nc, in_maps, list(range(NCORES)), **kw)
    acc = np.zeros((BT, C), dtype=np.float32)
    for i in range(NCORES):
        acc += np.asarray(res.results[i]["out"], dtype=np.float32)
    acc += np.asarray(bp, dtype=np.float32)[None, :]
    return acc.reshape(B, T, C), res


def kernel(x, Wq, Wk, Wv, Wp, bp):
    out, _ = run(x, Wq, Wk, Wv, Wp, bp)
    return out



# revision 4
# speedup vs baseline: 1.0281x; 1.0281x over previous
"""Multi-head causal attention (B=4, T=1024, C=1024, H=16, D=64) on 8 TRN2 cores.

Sharding: tensor-parallel over heads. Core i owns heads {2i, 2i+1}:
  - x is replicated (sent pre-transposed as xT [C, B*T], bf16)
  - Wq/Wk/Wv sharded over heads -> per-core [C, 128] (2 heads concat on D)
  - row-parallel output projection: per-core Wp rows [128, C]; host sums the
    8 partial [B*T, C] outputs (the all-reduce) and adds bp.

Step-1 scheduling rewrite over the 156us baseline:
  - x DMAs split per batch (b0 chunked for fast lead-in, b1-3 one 3D DMA
    each) so batch 1+ data lands before the PE needs it (the old bulk load
    stalled the PE 4us at b1 and re-throttled HAM to half clock for 17us)
  - v is computed transposed directly (x chunk stationary, Wv moving) ->
    no PE transposes, no psvt pool; the freed PSUM banks give the scores
    [128, 2, 512] tiles (both heads per tile)
  - exp merged: one ACTIVATE per (s, piece) covering both heads (48 instead
    of 96 instrs; each carries a 352-cycle pipeline-fill overhead)
  - denominators: DMA straight from PSUM ones-row to packed [16,2,32], DVE
    reciprocal, DRAM bounce, [64,512] broadcasts (no ACT den copies)
  - lag-1 proj pipeline (pop prev batch's halves during each batch) and
    per-tile 256KB out DMAs dispatched from GpSimd -> short tail drain
  - engine split: ACT=exp only, DVE=all psum copies+normalize muls+recip,
    GPSIMD=masks+out-DMA dispatch, Sync=in-DMAs+den bounce
"""

import ml_dtypes
import numpy as np

B, T, C = 4, 1024, 1024
H, D = 16, 64
NCORES = 8
HPC = H // NCORES      # heads per core = 2
D2 = HPC * D           # 128
BT = B * T
SCALE = 1.0 / np.sqrt(np.float32(C))  # 1/32
BF16 = ml_dtypes.bfloat16

_compiled = None

NWARM = 8


def _split_multi_waits(nc, mybir, maxw=1):
    """Walrus in this container encodes at most one sync wait per
    instruction (fp32 self-loading matmuls and drains overflow).  Hoist
    excess waits onto same-engine NoOps inserted just before."""
    for fn in nc.m.functions:
        for bb in fn.blocks:
            new = []
            for inst in bb.instructions:
                si = inst.sync_info
                waits = list(si.on_wait) if (si is not None and si.on_wait) else []
                if len(waits) > maxw:
                    extra, keep = waits[:-maxw], waits[-maxw:]
                    for j, w in enumerate(extra):
                        new.append(
                            mybir.InstNoOp(
                                name=f"{inst.name}-wsplit{j}",
                                engine=inst.engine,
                                sync_info=mybir.SyncInfo(on_wait=[w], on_update=[]),
                                bass_nofuse=True,
                            )
                        )
                    inst.sync_info = mybir.SyncInfo(
                        on_wait=keep,
                        on_update=list(si.on_update) if si.on_update else [],
                    )
                new.append(inst)
            bb.instructions = new


def _build():
    import concourse.bass as bass
    import concourse.mybir as mybir
    import concourse.tile as tile

    f32 = mybir.dt.float32
    bf = mybir.dt.bfloat16
    EXP = mybir.ActivationFunctionType.Exp

    nc = bass.Bass("TRN2", target_bir_lowering=False, debug=False, num_devices=NCORES)

    xT_d = nc.dram_tensor("xT", [C, BT], bf, kind="ExternalInput").ap()
    # host pre-shuffles each weight to [p, k, m] so the DMA is contiguous
    wq_d = nc.dram_tensor("wq", [128, C // 128, D2], bf, kind="ExternalInput").ap()
    wk_d = nc.dram_tensor("wk", [128, C // 128, D2], bf, kind="ExternalInput").ap()
    wv_d = nc.dram_tensor("wv", [128, C // 128, D2], bf, kind="ExternalInput").ap()
    wp_d = nc.dram_tensor("wp", [D2, C], bf, kind="ExternalInput").ap()
    mask_d = nc.dram_tensor("mask", [128, HPC, 128], bf, kind="ExternalInput").ap()
    out_d = nc.dram_tensor("out", [BT, C], bf, kind="ExternalOutput").ap()

    KC = C // 128  # 8 contraction chunks over C
    NS = T // 128  # 8 s-chunks

    import concourse.bass as _bass

    with tile.TileContext(nc) as tc:
        with (
            tc.tile_pool(name="const", bufs=1) as constp,
            tc.tile_pool(name="xin", bufs=1) as xinp,
            tc.tile_pool(name="qkv", bufs=2) as qkvp,
            tc.tile_pool(name="vaug", bufs=2) as vaugp,
            tc.tile_pool(name="exps", bufs=10) as expp,
            tc.tile_pool(name="smalls", bufs=2) as smallp,
            tc.tile_pool(name="outt", bufs=3) as outtp,
            tc.tile_pool(name="pout", bufs=3) as poutp,
            tc.tile_pool(name="dram", bufs=2, space="DRAM") as dramp,
            tc.tile_pool(name="psc", bufs=2, space="PSUM") as pscp,
            tc.tile_pool(name="psatt", bufs=2, space="PSUM") as psattp,
            tc.tile_pool(name="psproj", bufs=2, space="PSUM") as psprojp,
        ):
            # ---- constants / warmup ----
            wq_s = constp.tile([128, KC, D2], bf, tag="wq")
            wk_s = constp.tile([128, KC, D2], bf, tag="wk")
            wv_s = constp.tile([128, KC, D2], bf, tag="wv")
            wp_s = constp.tile([128, C], bf, tag="wp")
            mask_s = constp.tile([128, HPC, 128], bf, tag="mask")
            junk = constp.tile([128, 512], bf, tag="junk")

            # Warm-up: junk matmuls with no DMA deps fill the PE stream while
            # inputs land, so HAM un-throttles before real work.  memset on
            # GpSimd so it issues during the framework preamble.
            nc.gpsimd.memset(junk[:], 0.0)
            for i in range(NWARM):
                pw = pscp.tile([128, 2, 512], f32, tag="sc", name=f"warm{i}")
                nc.tensor.matmul(
                    pw[:, 0, :], junk[:, 0:128], junk[:], start=True, stop=True
                )

            # Input DMAs, critical-first: wq/wk + batch-0 x chunks gate the
            # first real matmuls; batches 1-3 land as one 3D DMA each.
            xba = xinp.tile([128, KC, BT], bf, tag="xba", name="xba", bufs=1)
            nc.sync.dma_start(wq_s[:], wq_d)
            nc.sync.dma_start(wk_s[:], wk_d)
            for k in range(KC):
                nc.sync.dma_start(
                    xba[:, k, 0:T], xT_d[k * 128:(k + 1) * 128, 0:T]
                )
                if k == 1:
                    nc.sync.dma_start(wv_s[:], wv_d)
                if k == 2:
                    nc.sync.dma_start(mask_s[:], mask_d)
                if k == 3:
                    nc.sync.dma_start(wp_s[:], wp_d)
            for b in range(1, B):
                nc.sync.dma_start(
                    xba[:, :, b * T:(b + 1) * T],
                    xT_d[:, b * T:(b + 1) * T].rearrange(
                        "(k p) t -> p k t", p=128
                    ),
                )

            def emit_qk(b, xb):
                """q and k for batch b, interleaved per k-chunk so the lead-in
                tracks the b0 chunk DMAs.  One [128,2,512] psum per half:
                q in [:,0,:], k in [:,1,:]."""
                qT = qkvp.tile([128, T], bf, tag="qT", name=f"qT{b}")
                kT = qkvp.tile([128, T], bf, tag="kT", name=f"kT{b}")
                for half in range(2):
                    ps = pscp.tile([128, 2, 512], f32, tag="sc",
                                   name=f"qk{b}_{half}")
                    cs = slice(half * 512, (half + 1) * 512)
                    for k in range(KC):
                        nc.tensor.matmul(
                            ps[:, 0, :], wq_s[:, k, :], xb[:, k, cs],
                            start=(k == 0), stop=(k == KC - 1),
                        )
                        nc.tensor.matmul(
                            ps[:, 1, :], wk_s[:, k, :], xb[:, k, cs],
                            start=(k == 0), stop=(k == KC - 1),
                        )
                    nc.vector.tensor_copy(qT[:, cs], ps[:, 0, :])
                    nc.vector.tensor_copy(kT[:, cs], ps[:, 1, :])
                return qT, kT

            def emit_vT(b, xb, vaug, srange):
                """v computed transposed: x chunk stationary [c,t128], Wv
                moving [c,128] -> psum [t128, 128(2 heads x 64)], accumulated
                over k; one DVE copy into vaug per t-chunk."""
                pv = psattp.tile([128, 512], f32, tag="att",
                                 name=f"vT{b}_{srange[0]}")
                for i, s in enumerate(srange):
                    for k in range(KC):
                        nc.tensor.matmul(
                            pv[:, i * 128:(i + 1) * 128],
                            xb[:, k, s * 128:(s + 1) * 128],
                            wv_s[:, k, :],
                            start=(k == 0), stop=(k == KC - 1),
                        )
                for i, s in enumerate(srange):
                    nc.vector.tensor_copy(
                        vaug[:, s, :, 0:64],
                        pv[:, i * 128:(i + 1) * 128].rearrange(
                            "p (h d) -> p h d", h=HPC
                        ),
                    )

            def emit_scores_s(b, s, qT, kT, exs):
                """Scores for chunk s, both heads.  ex layout: col j of
                ex[:,h,:] is t = s0 + j.  One exp ACTIVATE per piece covers
                both heads; diagonal-block mask is one GPSIMD op."""
                s0 = s * 128
                d1 = max(0, s0 - 512)
                ex = expp.tile([128, HPC, 1024], bf, tag="ex", bufs=10,
                               name=f"ex{b}_{s}")
                exs[s] = ex
                if s < 4:  # t-half0 piece: cols [s0, 512)
                    w0 = 512 - s0
                    pa = pscp.tile([128, 2, 512], f32, tag="sc",
                                   name=f"sc{b}_{s}a")
                    for h in range(HPC):
                        hp = slice(h * 64, (h + 1) * 64)
                        nc.tensor.matmul(
                            pa[:, h, 0:w0], kT[hp, s0:s0 + 128],
                            qT[hp, s0:512], start=True, stop=True,
                        )
                    nc.scalar.activation(
                        ex[:, :, 0:w0], pa[:, :, 0:w0], EXP, scale=float(SCALE)
                    )
                # t-half1 piece: cols [max(512, s0), 1024)
                w1 = 512 - d1
                pb = pscp.tile([128, 2, 512], f32, tag="sc",
                               name=f"sc{b}_{s}b")
                for h in range(HPC):
                    hp = slice(h * 64, (h + 1) * 64)
                    nc.tensor.matmul(
                        pb[:, h, 0:w1], kT[hp, s0:s0 + 128],
                        qT[hp, 512 + d1:T], start=True, stop=True,
                    )
                nc.scalar.activation(
                    ex[:, :, 512 - s0 + d1:T - s0], pb[:, :, 0:w1],
                    EXP, scale=float(SCALE),
                )
                nc.gpsimd.tensor_mul(ex[:, :, 0:128], ex[:, :, 0:128], mask_s[:])

            def emit_po0_s(b, s, vaug, exs, po0):
                assert 0 <= s <= 3
                s0 = s * 128
                for h in range(HPC):
                    nc.tensor.matmul(
                        po0[h][0:65, s0:512],
                        vaug[:, s, h, 0:65],
                        exs[s][:, h, 0:512 - s0],
                        start=(s == 0),
                        stop=(s == 3),
                    )

            def emit_den(b, half, den_srcs, scr_rec):
                """DMA the psum ones-rows to packed [16,2,32], DVE recip,
                bounce through DRAM for contiguous broadcast source."""
                den2 = smallp.tile([1, HPC, 512], f32, tag="den2", bufs=2,
                                   name=f"dn{b}_{half}")
                nc.scalar.copy(den2[:, 0, :], den_srcs[0])
                nc.vector.tensor_copy(den2[:, 1, :], den_srcs[1])
                packed = smallp.tile([16, HPC, 32], f32, tag="packed",
                                     name=f"pk{b}_{half}")
                nc.sync.dma_start(packed[:], den2[:])
                recp = smallp.tile([16, HPC, 32], f32, tag="recp",
                                   name=f"rc{b}_{half}")
                nc.vector.reciprocal(recp[:], packed[:])
                nc.sync.dma_start(
                    scr_rec[0, :].rearrange("(p h f) -> p h f", p=16, h=HPC),
                    recp[:],
                )

            def emit_norm_half(b, half, po_h, outT2, scr_rec):
                """Broadcast each head's reciprocals to [64,512] and apply."""
                t0 = half * 512
                for h in range(HPC):
                    hp = slice(h * 64, (h + 1) * 64)
                    rec2 = smallp.tile([64, 512], f32, tag="rec2", bufs=4,
                                       name=f"rec2_{b}_{half}_{h}")
                    nc.sync.dma_start(
                        rec2[:],
                        _bass.AP(
                            scr_rec[:].tensor,
                            scr_rec[:].offset + 512 * h,
                            [[0, 64], [1, 512]],
                        ),
                    )
                    nc.vector.tensor_mul(
                        outT2[hp, t0:t0 + 512], po_h[h][0:64, 0:512], rec2[:]
                    )

            def emit_proj_tile(pb, o2, i, tt, ob, pool_tag="proj"):
                for ct in range(2):
                    pp = psprojp.tile([128, 512], f32, tag=pool_tag,
                                      name=f"pj{pb}_{tt}_{ct}")
                    nc.tensor.matmul(
                        pp[:],
                        o2[:, tt * 128:(tt + 1) * 128],
                        wp_s[:, ct * 512:(ct + 1) * 512],
                        start=True, stop=True,
                    )
                    nc.vector.tensor_copy(
                        ob[:, i, ct * 512:(ct + 1) * 512], pp[:]
                    )
                # per-tile 256KB out DMA, dispatched from GpSimd
                r0 = pb * T + (tt // 4) * 512 + i * 128
                nc.gpsimd.dma_start(out_d[r0:r0 + 128, :], ob[:, i, :])

            def proj_half_ob(pb, half):
                return poutp.tile([128, 4, C], bf, tag="ob",
                                  name=f"ob{pb}_{half}")

            # ---- main pipeline ----
            pend = []  # pending proj halves: (batch, outT2, half)

            def pop_proj(b, lag=1):
                if pend and pend[0][0] <= b - lag:
                    pb, o2, half = pend.pop(0)
                    ob = proj_half_ob(pb, half)

                    def mk(i, tt):
                        def thunk():
                            emit_proj_tile(pb, o2, i, tt, ob)
                        return thunk

                    return [
                        mk(i, tt)
                        for i, tt in enumerate(range(half * 4, half * 4 + 4))
                    ]
                return []

            for b in range(B):
                xb = xba[:, :, b * T:(b + 1) * T]
                vaug = vaugp.tile([128, NS, HPC, 66], bf, tag="vaug",
                                  name=f"vaug{b}")
                nc.vector.memset(vaug[:, :, :, 64:65], 1.0)
                outT2 = outtp.tile([128, T], bf, tag="outT2", name=f"outT2_{b}")
                exs = {}
                scr0 = dramp.tile([1, 1024], f32, tag="scr", name=f"scr{b}_0")
                scr1 = dramp.tile([1, 1024], f32, tag="scr2", name=f"scr{b}_1")

                qT, kT = emit_qk(b, xb)
                # scores s=0 early: ACT exp head start over the vT PE work.
                emit_scores_s(b, 0, qT, kT, exs)
                emit_vT(b, xb, vaug, range(0, 4))
                emit_scores_s(b, 1, qT, kT, exs)
                emit_vT(b, xb, vaug, range(4, NS))

                slot_a = pop_proj(b)  # prev batch half0, spread into s-loop
                po0 = [
                    psattp.tile([128, 512], f32, tag="att", name=f"po0_{b}_{h}")
                    for h in range(HPC)
                ]
                for s in range(2, NS):
                    emit_scores_s(b, s, qT, kT, exs)
                    if s - 2 <= 3:
                        emit_po0_s(b, s - 2, vaug, exs, po0)
                    if s - 2 == 3:
                        # po0 complete -> launch half0 denominator chain
                        emit_den(b, 0, [po0[h][64:65, 0:512] for h in range(HPC)],
                                 scr0)
                    if s == 6:
                        emit_norm_half(b, 0, po0, outT2, scr0)
                        pend.append((b, outT2, 0))
                    if 0 <= s - 3 < len(slot_a):
                        slot_a[s - 3]()  # PE backfill, no exp dep

                # po1: one [128,2,512] scores-pool tile, h on middle dim
                po1t = pscp.tile([128, 2, 512], f32, tag="sc", name=f"po1_{b}")
                po1 = [po1t[:, h, :] for h in range(HPC)]
                slot_b = pop_proj(b)
                for h in range(HPC):
                    for s in range(NS):
                        s0 = s * 128
                        d1 = max(0, s0 - 512)
                        nc.tensor.matmul(
                            po1[h][0:65, d1:512],
                            vaug[:, s, h, 0:65],
                            exs[s][:, h, 512 - s0 + d1:T - s0],
                            start=(s == 0),
                            stop=(s == NS - 1),
                        )
                    if h == 0:
                        for t in slot_b[:2]:
                            t()
                emit_den(b, 1, [po1[h][64:65, 0:512] for h in range(HPC)], scr1)
                for t in slot_b[2:]:
                    t()
                if b == B - 1:
                    # flush own half0 while the half1 den chain bounces
                    for t in pop_proj(b, lag=0):
                        t()
                emit_norm_half(b, 1, po1, outT2, scr1)
                pend.append((b, outT2, 1))

            while pend:
                pb, o2, half = pend.pop(0)
                ob = proj_half_ob(pb, half)
                for i, tt in enumerate(range(half * 4, half * 4 + 4)):
                    emit_proj_tile(pb, o2, i, tt, ob)

    _split_multi_waits(nc, mybir)
    return nc


def _get_compiled():
    global _compiled
    if _compiled is None:
        _compiled = _build()
    return _compiled


def _shuf_w(W, h0):
    # [H, C, D] head-pair slice -> [C, D2] -> pre-shuffled [p, k, m] so the
    # device DMA is one contiguous [128, 1024] transfer per weight.
    w = np.asarray(W[h0:h0 + HPC], dtype=np.float32).transpose(1, 0, 2).reshape(C, D2)
    return np.ascontiguousarray(
        w.reshape(C // 128, 128, D2).transpose(1, 0, 2)
    ).astype(BF16)


def _make_in_maps(x, Wq, Wk, Wv, Wp):
    xT = np.ascontiguousarray(
        np.asarray(x, dtype=np.float32).reshape(BT, C).T
    ).astype(BF16)  # [C, BT]
    mask1 = np.triu(np.ones((128, 128), dtype=BF16))  # keep j >= i
    mask = np.ascontiguousarray(
        np.stack([mask1] * HPC, axis=1)
    )  # [128, HPC, 128]
    in_maps = []
    for i in range(NCORES):
        h0 = i * HPC
        wp = np.ascontiguousarray(
            np.asarray(Wp, dtype=np.float32)[h0 * D:(h0 + HPC) * D, :]
        ).astype(BF16)
        in_maps.append(
            {"xT": xT, "wq": _shuf_w(Wq, h0), "wk": _shuf_w(Wk, h0),
             "wv": _shuf_w(Wv, h0), "wp": wp, "mask": mask}
        )
    return in_maps


def run(x, Wq, Wk, Wv, Wp, bp, trace=False, trace_cores=None):
    """Returns (full_output [B,T,C], BassKernelResults)."""
    from concourse.bass_utils import run_bass_kernel_spmd

    nc = _get_compiled()
    in_maps = _make_in_maps(x, Wq, Wk, Wv, Wp)
    kw = {}
    if trace:
        kw = {"trace": True, "trace_cores": trace_cores or [0]}
    res = run_bass_kernel_spmd(nc, in_maps, list(range(NCORES)), **kw)
    acc = np.zeros((BT, C), dtype=np.float32)
    for i in range(NCORES):
        acc += np.asarray(res.results[i]["out"], dtype=np.float32)
    acc += np.asarray(bp, dtype=np.float32)[None, :]
    return acc.reshape(B, T, C), res


def kernel(x, Wq, Wk, Wv, Wp, bp):
    out, _ = run(x, Wq, Wk, Wv, Wp, bp)
    return out


# revision 11
# speedup vs baseline: 1.0960x; 1.0661x over previous
"""Multi-head causal attention (B=4, T=1024, C=1024, H=16, D=64) on 8 TRN2 cores.

Sharding: tensor-parallel over heads. Core i owns heads {2i, 2i+1}:
  - x is replicated (sent pre-transposed as xT [C, B*T], bf16)
  - Wq/Wk/Wv sharded over heads -> per-core [C, 128] (2 heads concat on D)
  - row-parallel output projection: per-core Wp rows [128, C]; host sums the
    8 partial [B*T, C] outputs (the all-reduce) and adds bp.

Step-1 scheduling rewrite over the 156us baseline:
  - x DMAs split per batch (b0 chunked for fast lead-in, b1-3 one 3D DMA
    each) so batch 1+ data lands before the PE needs it (the old bulk load
    stalled the PE 4us at b1 and re-throttled HAM to half clock for 17us)
  - v is computed transposed directly (x chunk stationary, Wv moving) ->
    no PE transposes, no psvt pool; the freed PSUM banks give the scores
    [128, 2, 512] tiles (both heads per tile)
  - exp merged: one ACTIVATE per (s, piece) covering both heads (48 instead
    of 96 instrs; each carries a 352-cycle pipeline-fill overhead)
  - denominators: DMA straight from PSUM ones-row to packed [16,2,32], DVE
    reciprocal, DRAM bounce, [64,512] broadcasts (no ACT den copies)
  - lag-1 proj pipeline (pop prev batch's halves during each batch) and
    per-tile 256KB out DMAs dispatched from GpSimd -> short tail drain
  - engine split: ACT=exp only, DVE=all psum copies+normalize muls+recip,
    GPSIMD=masks+out-DMA dispatch, Sync=in-DMAs+den bounce
"""

import ml_dtypes
import numpy as np

B, T, C = 4, 1024, 1024
H, D = 16, 64
NCORES = 8
HPC = H // NCORES      # heads per core = 2
D2 = HPC * D           # 128
BT = B * T
SCALE = 1.0 / np.sqrt(np.float32(C))  # 1/32
BF16 = ml_dtypes.bfloat16

_compiled = None

NWARM = 12


def _split_multi_waits(nc, mybir, maxw=1):
    """Walrus in this container encodes at most one sync wait per
    instruction (fp32 self-loading matmuls and drains overflow).  Hoist
    excess waits onto same-engine NoOps inserted just before."""
    for fn in nc.m.functions:
        for bb in fn.blocks:
            new = []
            for inst in bb.instructions:
                si = inst.sync_info
                waits = list(si.on_wait) if (si is not None and si.on_wait) else []
                if len(waits) > maxw:
                    extra, keep = waits[:-maxw], waits[-maxw:]
                    for j, w in enumerate(extra):
                        new.append(
                            mybir.InstNoOp(
                                name=f"{inst.name}-wsplit{j}",
                                engine=inst.engine,
                                sync_info=mybir.SyncInfo(on_wait=[w], on_update=[]),
                                bass_nofuse=True,
                            )
                        )
                    inst.sync_info = mybir.SyncInfo(
                        on_wait=keep,
                        on_update=list(si.on_update) if si.on_update else [],
                    )
                new.append(inst)
            bb.instructions = new


def _build():
    import concourse.bass as bass
    import concourse.mybir as mybir
    import concourse.tile as tile

    f32 = mybir.dt.float32
    bf = mybir.dt.bfloat16
    EXP = mybir.ActivationFunctionType.Exp

    nc = bass.Bass("TRN2", target_bir_lowering=False, debug=False, num_devices=NCORES)

    xT_d = nc.dram_tensor("xT", [C, BT], bf, kind="ExternalInput").ap()
    # host pre-shuffles each weight to [p, k, m] so the DMA is contiguous
    wq_d = nc.dram_tensor("wq", [128, C // 128, D2], bf, kind="ExternalInput").ap()
    wk_d = nc.dram_tensor("wk", [128, C // 128, D2], bf, kind="ExternalInput").ap()
    wv_d = nc.dram_tensor("wv", [128, C // 128, D2], bf, kind="ExternalInput").ap()
    wp_d = nc.dram_tensor("wp", [D2, C], bf, kind="ExternalInput").ap()
    mask_d = nc.dram_tensor("mask", [128, HPC, 128], bf, kind="ExternalInput").ap()
    out_d = nc.dram_tensor("out", [BT, C], bf, kind="ExternalOutput").ap()

    KC = C // 128  # 8 contraction chunks over C
    NS = T // 128  # 8 s-chunks

    import concourse.bass as _bass

    with tile.TileContext(nc) as tc:
        with (
            tc.tile_pool(name="const", bufs=1) as constp,
            tc.tile_pool(name="xin", bufs=1) as xinp,
            tc.tile_pool(name="qkv", bufs=2) as qkvp,
            tc.tile_pool(name="vaug", bufs=2) as vaugp,
            tc.tile_pool(name="exps", bufs=10) as expp,
            tc.tile_pool(name="smalls", bufs=2) as smallp,
            tc.tile_pool(name="outt", bufs=3) as outtp,
            tc.tile_pool(name="pout", bufs=3) as poutp,
            tc.tile_pool(name="dram", bufs=2, space="DRAM") as dramp,
            tc.tile_pool(name="psc", bufs=2, space="PSUM") as pscp,
            tc.tile_pool(name="psatt", bufs=2, space="PSUM") as psattp,
            tc.tile_pool(name="psproj", bufs=1, space="PSUM") as psprojp,
        ):
            # ---- constants / warmup ----
            wq_s = constp.tile([128, KC, D2], bf, tag="wq")
            wk_s = constp.tile([128, KC, D2], bf, tag="wk")
            wv_s = constp.tile([128, KC, D2], bf, tag="wv")
            wp_s = constp.tile([128, C], bf, tag="wp")
            mask_s = constp.tile([128, HPC, 128], bf, tag="mask")
            junk = constp.tile([128, 512], bf, tag="junk")

            # Warm-up: junk matmuls with no DMA deps fill the PE stream while
            # inputs land, so HAM un-throttles before real work.  memset on
            # GpSimd so it issues during the framework preamble.
            nc.gpsimd.memset(junk[:], 0.0)
            for i in range(NWARM):
                pw = pscp.tile([128, 2, 512], f32, tag="sc", name=f"warm{i}")
                nc.tensor.matmul(
                    pw[:, 0, :], junk[:, 0:128], junk[:], start=True, stop=True
                )

            # Input DMAs, critical-first: wq/wk + batch-0 x chunks gate the
            # first real matmuls; batches 1-3 land as one 3D DMA each.
            xba = xinp.tile([128, KC, BT], bf, tag="xba", name="xba", bufs=1)
            nc.sync.dma_start(wq_s[:], wq_d)
            nc.sync.dma_start(wk_s[:], wk_d)
            for k in range(KC):
                nc.sync.dma_start(
                    xba[:, k, 0:T], xT_d[k * 128:(k + 1) * 128, 0:T]
                )
                if k == 1:
                    nc.sync.dma_start(wv_s[:], wv_d)
                if k == 2:
                    nc.sync.dma_start(mask_s[:], mask_d)
                if k == 3:
                    nc.sync.dma_start(wp_s[:], wp_d)
            for b in range(1, B):
                nc.sync.dma_start(
                    xba[:, :, b * T:(b + 1) * T],
                    xT_d[:, b * T:(b + 1) * T].rearrange(
                        "(k p) t -> p k t", p=128
                    ),
                )

            def emit_qk(b, xb):
                """q and k for batch b, interleaved per k-chunk so the lead-in
                tracks the b0 chunk DMAs.  One [128,2,512] psum per half:
                q in [:,0,:], k in [:,1,:]; one merged copy per half."""
                qkT = qkvp.tile([128, 2, T], bf, tag="qkT", name=f"qkT{b}")
                for half in range(2):
                    ps = pscp.tile([128, 2, 512], f32, tag="sc",
                                   name=f"qk{b}_{half}")
                    cs = slice(half * 512, (half + 1) * 512)
                    for k in range(KC):
                        nc.tensor.matmul(
                            ps[:, 0, :], wq_s[:, k, :], xb[:, k, cs],
                            start=(k == 0), stop=(k == KC - 1),
                        )
                        nc.tensor.matmul(
                            ps[:, 1, :], wk_s[:, k, :], xb[:, k, cs],
                            start=(k == 0), stop=(k == KC - 1),
                        )
                    nc.vector.tensor_copy(qkT[:, :, cs], ps[:])
                return qkT[:, 0, :], qkT[:, 1, :]

            def emit_vT(b, xb, vaug, srange):
                """v computed transposed: x chunk stationary [c,t128], Wv
                moving [c,128] -> psum [t128, 128(2 heads x 64)], accumulated
                over k; one DVE copy into vaug per t-chunk."""
                pv = psattp.tile([128, 512], f32, tag="att",
                                 name=f"vT{b}_{srange[0]}")
                for i, s in enumerate(srange):
                    for k in range(KC):
                        nc.tensor.matmul(
                            pv[:, i * 128:(i + 1) * 128],
                            xb[:, k, s * 128:(s + 1) * 128],
                            wv_s[:, k, :],
                            start=(k == 0), stop=(k == KC - 1),
                        )
                nc.vector.tensor_copy(
                    vaug[:, srange[0]:srange[0] + 4, :, 0:64],
                    pv[:].rearrange("p (s h d) -> p s h d", s=4, h=HPC),
                )

            def emit_scores_s(b, s, qT, kT, exs):
                """Scores for chunk s, both heads.  ex layout: col j of
                ex[:,h,:] is t = s0 + j.  One exp ACTIVATE per piece covers
                both heads; diagonal-block mask is one GPSIMD op."""
                s0 = s * 128
                d1 = max(0, s0 - 512)
                ex = expp.tile([128, HPC, 1024], bf, tag="ex", bufs=10,
                               name=f"ex{b}_{s}")
                exs[s] = ex
                if s < 4:  # t-half0 piece: cols [s0, 512)
                    w0 = 512 - s0
                    pa = pscp.tile([128, 2, 512], f32, tag="sc",
                                   name=f"sc{b}_{s}a")
                    for h in range(HPC):
                        hp = slice(h * 64, (h + 1) * 64)
                        nc.tensor.matmul(
                            pa[:, h, 0:w0], kT[hp, s0:s0 + 128],
                            qT[hp, s0:512], start=True, stop=True,
                        )
                    nc.scalar.activation(
                        ex[:, :, 0:w0], pa[:, :, 0:w0], EXP, scale=float(SCALE)
                    )
                # t-half1 piece: cols [max(512, s0), 1024)
                w1 = 512 - d1
                pb = pscp.tile([128, 2, 512], f32, tag="sc",
                               name=f"sc{b}_{s}b")
                for h in range(HPC):
                    hp = slice(h * 64, (h + 1) * 64)
                    nc.tensor.matmul(
                        pb[:, h, 0:w1], kT[hp, s0:s0 + 128],
                        qT[hp, 512 + d1:T], start=True, stop=True,
                    )
                nc.scalar.activation(
                    ex[:, :, 512 - s0 + d1:T - s0], pb[:, :, 0:w1],
                    EXP, scale=float(SCALE),
                )
                nc.gpsimd.tensor_mul(ex[:, :, 0:128], ex[:, :, 0:128], mask_s[:])

            def emit_po0_s(b, s, vaug, exs, po0):
                assert 0 <= s <= 3
                s0 = s * 128
                for h in range(HPC):
                    nc.tensor.matmul(
                        po0[h][0:65, s0:512],
                        vaug[:, s, h, 0:65],
                        exs[s][:, h, 0:512 - s0],
                        start=(s == 0),
                        stop=(s == 3),
                    )

            def emit_den(b, half, den_srcs, scr_rec):
                """DMA the psum ones-rows to packed [16,2,32], DVE recip,
                bounce through DRAM for contiguous broadcast source."""
                den2 = smallp.tile([1, HPC, 512], f32, tag="den2", bufs=2,
                                   name=f"dn{b}_{half}")
                nc.scalar.copy(den2[:, 0, :], den_srcs[0])
                nc.vector.tensor_copy(den2[:, 1, :], den_srcs[1])
                packed = smallp.tile([16, HPC, 32], f32, tag="packed",
                                     name=f"pk{b}_{half}")
                nc.sync.dma_start(packed[:], den2[:])
                recp = smallp.tile([16, HPC, 32], f32, tag="recp",
                                   name=f"rc{b}_{half}")
                nc.vector.reciprocal(recp[:], packed[:])
                nc.sync.dma_start(
                    scr_rec[0, :].rearrange("(p h f) -> p h f", p=16, h=HPC),
                    recp[:],
                )

            def emit_norm_half(b, half, po_h, outT2, scr_rec):
                """Broadcast each head's reciprocals to [64,512] and apply."""
                t0 = half * 512
                for h in range(HPC):
                    hp = slice(h * 64, (h + 1) * 64)
                    rec2 = smallp.tile([64, 512], f32, tag="rec2", bufs=4,
                                       name=f"rec2_{b}_{half}_{h}")
                    nc.sync.dma_start(
                        rec2[:],
                        _bass.AP(
                            scr_rec[:].tensor,
                            scr_rec[:].offset + 512 * h,
                            [[0, 64], [1, 512]],
                        ),
                    )
                    nc.vector.tensor_mul(
                        outT2[hp, t0:t0 + 512], po_h[h][0:64, 0:512], rec2[:]
                    )

            def emit_proj_tile(pb, o2, i, tt, ob, flush=None):
                """One [128,2,512] psum per proj tile: both 512-col halves of
                Wp accumulate side by side, then ONE merged [128,1024] cast.
                flush='act'/'alt' uses the (idle) scores-pool banks and routes
                casts to ACT / alternating engines for the end-of-kernel
                drain; default uses the single-buffer proj bank with DVE
                casts (1-in-4 on ACT)."""
                if flush is None:
                    pp = psprojp.tile([128, 2, 512], f32, tag="proj",
                                      name=f"pj{pb}_{tt}")
                    eng = "s" if i % 4 == 3 else "v"
                else:
                    pp = pscp.tile([128, 2, 512], f32, tag="sc",
                                   name=f"pjf{pb}_{tt}")
                    eng = "s" if (flush == "act" or i % 2 == 1) else "v"
                for ct in range(2):
                    nc.tensor.matmul(
                        pp[:, ct, :],
                        o2[:, tt * 128:(tt + 1) * 128],
                        wp_s[:, ct * 512:(ct + 1) * 512],
                        start=True, stop=True,
                    )
                if eng == "s":
                    nc.scalar.copy(ob[:, i, :], pp[:])
                else:
                    nc.vector.tensor_copy(ob[:, i, :], pp[:])
                # per-tile 256KB out DMA, dispatched from GpSimd
                r0 = pb * T + (tt // 4) * 512 + i * 128
                nc.gpsimd.dma_start(out_d[r0:r0 + 128, :], ob[:, i, :])

            def proj_half_ob(pb, half):
                return poutp.tile([128, 4, C], bf, tag="ob",
                                  name=f"ob{pb}_{half}")

            # ---- main pipeline ----
            pend = []  # pending proj halves: (batch, outT2, half)

            def pop_proj(b, lag=1, flush=None):
                if pend and pend[0][0] <= b - lag:
                    pb, o2, half = pend.pop(0)
                    ob = proj_half_ob(pb, half)

                    def mk(i, tt):
                        def thunk():
                            emit_proj_tile(pb, o2, i, tt, ob, flush=flush)
                        return thunk

                    return [
                        mk(i, tt)
                        for i, tt in enumerate(range(half * 4, half * 4 + 4))
                    ]
                return []

            for b in range(B):
                xb = xba[:, :, b * T:(b + 1) * T]
                vaug = vaugp.tile([128, NS, HPC, 66], bf, tag="vaug",
                                  name=f"vaug{b}")
                nc.vector.memset(vaug[:, :, :, 64:65], 1.0)
                outT2 = outtp.tile([128, T], bf, tag="outT2", name=f"outT2_{b}")
                exs = {}
                scr0 = dramp.tile([1, 1024], f32, tag="scr", name=f"scr{b}_0")
                scr1 = dramp.tile([1, 1024], f32, tag="scr2", name=f"scr{b}_1")

                qT, kT = emit_qk(b, xb)
                # scores s=0 early: ACT exp head start over the vT PE work.
                emit_scores_s(b, 0, qT, kT, exs)
                emit_vT(b, xb, vaug, range(0, 4))
                emit_scores_s(b, 1, qT, kT, exs)
                emit_vT(b, xb, vaug, range(4, NS))

                slot_a = pop_proj(b)  # prev batch half0, spread into s-loop
                po0 = [
                    psattp.tile([128, 512], f32, tag="att", name=f"po0_{b}_{h}")
                    for h in range(HPC)
                ]
                for s in range(2, NS):
                    emit_scores_s(b, s, qT, kT, exs)
                    if s - 2 <= 3:
                        emit_po0_s(b, s - 2, vaug, exs, po0)
                    if s - 2 == 3:
                        # po0 complete -> launch half0 denominator chain
                        emit_den(b, 0, [po0[h][64:65, 0:512] for h in range(HPC)],
                                 scr0)
                    if s == 6:
                        emit_norm_half(b, 0, po0, outT2, scr0)
                        pend.append((b, outT2, 0))
                    if 0 <= s - 3 < len(slot_a):
                        slot_a[s - 3]()  # PE backfill, no exp dep

                # po1: one [128,2,512] scores-pool tile, h on middle dim
                po1t = pscp.tile([128, 2, 512], f32, tag="sc", name=f"po1_{b}")
                po1 = [po1t[:, h, :] for h in range(HPC)]
                slot_b = pop_proj(b)
                for h in range(HPC):
                    for s in range(NS):
                        s0 = s * 128
                        d1 = max(0, s0 - 512)
                        nc.tensor.matmul(
                            po1[h][0:65, d1:512],
                            vaug[:, s, h, 0:65],
                            exs[s][:, h, 512 - s0 + d1:T - s0],
                            start=(s == 0),
                            stop=(s == NS - 1),
                        )
                    if h == 0:
                        for t in slot_b[:2]:
                            t()
                emit_den(b, 1, [po1[h][64:65, 0:512] for h in range(HPC)], scr1)
                # Evacuate po1 to SBUF right away: the sc-ring reuse otherwise
                # blocks the NEXT batch's qk psums behind this batch's
                # normalize bounce (the ~1.5-4us PE gap at every batch edge).
                po1s = smallp.tile([65, HPC, 512], bf, tag="po1s", bufs=2,
                                   name=f"po1s{b}")
                nc.vector.tensor_copy(po1s[:], po1t[0:65, :, :])
                for t in slot_b[2:]:
                    t()
                if b == B - 1:
                    # flush own half0 (casts on ACT so DVE reaches the half1
                    # normalize muls the moment the bounce lands)
                    for t in pop_proj(b, lag=0, flush="act"):
                        t()
                emit_norm_half(b, 1, [po1s[:, h, :] for h in range(HPC)],
                               outT2, scr1)
                pend.append((b, outT2, 1))

            while pend:
                pb, o2, half = pend.pop(0)
                ob = proj_half_ob(pb, half)
                for i, tt in enumerate(range(half * 4, half * 4 + 4)):
                    emit_proj_tile(pb, o2, i, tt, ob, flush="alt")

    _split_multi_waits(nc, mybir)
    return nc


def _get_compiled():
    global _compiled
    if _compiled is None:
        _compiled = _build()
    return _compiled


def _shuf_w(W, h0):
    # [H, C, D] head-pair slice -> [C, D2] -> pre-shuffled [p, k, m] so the
    # device DMA is one contiguous [128, 1024] transfer per weight.
    w = np.asarray(W[h0:h0 + HPC], dtype=np.float32).transpose(1, 0, 2).reshape(C, D2)
    return np.ascontiguousarray(
        w.reshape(C // 128, 128, D2).transpose(1, 0, 2)
    ).astype(BF16)


def _make_in_maps(x, Wq, Wk, Wv, Wp):
    xT = np.ascontiguousarray(
        np.asarray(x, dtype=np.float32).reshape(BT, C).T
    ).astype(BF16)  # [C, BT]
    mask1 = np.triu(np.ones((128, 128), dtype=BF16))  # keep j >= i
    mask = np.ascontiguousarray(
        np.stack([mask1] * HPC, axis=1)
    )  # [128, HPC, 128]
    in_maps = []
    for i in range(NCORES):
        h0 = i * HPC
        wp = np.ascontiguousarray(
            np.asarray(Wp, dtype=np.float32)[h0 * D:(h0 + HPC) * D, :]
        ).astype(BF16)
        in_maps.append(
            {"xT": xT, "wq": _shuf_w(Wq, h0), "wk": _shuf_w(Wk, h0),
             "wv": _shuf_w(Wv, h0), "wp": wp, "mask": mask}
        )
    return in_maps


def run(x, Wq, Wk, Wv, Wp, bp, trace=False, trace_cores=None):
    """Returns (full_output [B,T,C], BassKernelResults)."""
    from concourse.bass_utils import run_bass_kernel_spmd

    nc = _get_compiled()
    in_maps = _make_in_maps(x, Wq, Wk, Wv, Wp)
    kw = {}
    if trace:
        kw = {"trace": True, "trace_cores": trace_cores or [0]}
    res = run_bass_kernel_spmd(nc, in_maps, list(range(NCORES)), **kw)
    acc = np.zeros((BT, C), dtype=np.float32)
    for i in range(NCORES):
        acc += np.asarray(res.results[i]["out"], dtype=np.float32)
    acc += np.asarray(bp, dtype=np.float32)[None, :]
    return acc.reshape(B, T, C), res


def kernel(x, Wq, Wk, Wv, Wp, bp):
    out, _ = run(x, Wq, Wk, Wv, Wp, bp)
    return out
